# revision 1
# baseline (speedup 1.0000x reference)
"""Trainium2 Bass kernel for LocalPPFTransformer (sparse attention).

Strategy (data-parallel over M across 8 cores, feats replicated):
  Host folds every pre-attention linear op:
    k = feats@(W_in@Wk), v = feats@(W_in@Wv), q = feats@(W_in@Wq)*0.25
    p = ppfs@(W_embed@Wp), vp = ppfs@(W_embed@Wvp)
  Key/positional biases drop out of softmax (constant per head); value-side
  biases pass through softmax (sum attn = 1) and fold into the x bias.
  LayerNorm folds into y = x@diag(gamma)@Wout with per-row rescale.

  Device per 128-query tile:
    - indirect-DMA gathers of feats rows (bf16 for k/v path, f32 for q/resid)
    - PE transposes gathered tiles; fused [g^T; ppfs^T] @ [Wk|Wv; Wp|Wvp]
      accumulates kp/vvp in PSUM
    - DVE attention core: big strided-AP mul + segmented tensor_reduce ops,
      softmax without max subtraction (|scores| << 1 for this distribution)
    - folded LN + output matmul, DMA out
"""

import numpy as np
import ml_dtypes

import concourse.bass as bass
import concourse.bacc as bacc
import concourse.tile as tile
from concourse import mybir
from concourse.bass_utils import run_bass_kernel_spmd

BF16 = ml_dtypes.bfloat16

N, M, K = 50000, 20000, 32
IN_DIM, D, OUT_DIM, H = 64, 128, 128, 8
DH = D // H
EPS = 1e-5
NCORES = 8
MS = M // NCORES          # 2500 queries per core
P = 128                   # partitions / tile query count
TILES = (MS + P - 1) // P  # 20 tiles (last overlaps)
NIDX = TILES * (K + 1)    # idx columns per core: 32 group + 1 node per tile

_BUILD_CACHE = {}


def _tile_rows(t):
    start = t * P
    if start + P > MS:
        start = MS - P
    return start


def _build_nc():
    if "nc" in _BUILD_CACHE:
        return _BUILD_CACHE["nc"]

    f32 = mybir.dt.float32
    bf16 = mybir.dt.bfloat16
    i32 = mybir.dt.int32

    nc = bacc.Bacc()

    feats_bf = nc.declare_dram_parameter("feats_bf", [N, IN_DIM], bf16, isOutput=False)
    feats_f32 = nc.declare_dram_parameter("feats_f32", [N, IN_DIM], f32, isOutput=False)
    gidx = nc.declare_dram_parameter("gidx", [P, NIDX], i32, isOutput=False)
    # ppfs transposed: [tile, 4 coords, 8 quads * 512] bf16
    ppfs_t = nc.declare_dram_parameter("ppfs_t", [TILES, 4, K * P], bf16, isOutput=False)
    wkv = nc.declare_dram_parameter("wkv", [IN_DIM, 2 * D], bf16, isOutput=False)
    wpv = nc.declare_dram_parameter("wpv", [4, 2 * D], bf16, isOutput=False)
    wqi = nc.declare_dram_parameter("wqi", [IN_DIM, 2 * D], f32, isOutput=False)
    wl = nc.declare_dram_parameter("wl", [D, D], bf16, isOutput=False)
    wg = nc.declare_dram_parameter("wg", [D, D], bf16, isOutput=False)
    bq_rep = nc.declare_dram_parameter("bq_rep", [P, D], f32, isOutput=False)
    ball_rep = nc.declare_dram_parameter("ball_rep", [P, D], f32, isOutput=False)
    gwbo = nc.declare_dram_parameter("gwbo", [P, 2 * D], f32, isOutput=False)
    id_bf = nc.declare_dram_parameter("id_bf", [P, P], bf16, isOutput=False)
    id_f32 = nc.declare_dram_parameter("id_f32", [P, P], f32, isOutput=False)
    out = nc.declare_dram_parameter("out", [MS, OUT_DIM], f32, isOutput=True)

    AX = mybir.AxisListType
    ALU = mybir.AluOpType
    ACT_F = mybir.ActivationFunctionType

    with tile.TileContext(nc) as tc:
        with (
            tc.tile_pool(name="const", bufs=1) as cpool,
            tc.tile_pool(name="gq", bufs=6) as gqp,
            tc.tile_pool(name="gaug", bufs=3) as gaugp,
            tc.tile_pool(name="kpv_sb", bufs=2) as kpvsbp,
            tc.tile_pool(name="prod", bufs=2) as prodp,
            tc.tile_pool(name="attn_sm", bufs=2) as smp,
            tc.tile_pool(name="post", bufs=2) as postp,
            tc.tile_pool(name="tr_ps", bufs=2, space="PSUM") as trps,
            tc.tile_pool(name="kpv_ps", bufs=2, space="PSUM") as kpvps,
            tc.tile_pool(name="qres_ps", bufs=2, space="PSUM") as qresps,
            tc.tile_pool(name="y_ps", bufs=1, space="PSUM") as yps,
        ):
            # ---- static loads ----
            idx_sb = cpool.tile([P, NIDX], i32)
            nc.sync.dma_start(out=idx_sb[:], in_=gidx[:])
            wkv_sb = cpool.tile([IN_DIM, 2 * D], bf16)
            nc.sync.dma_start(out=wkv_sb[:], in_=wkv[:])
            wpv_sb = cpool.tile([4, 2 * D], bf16)
            nc.sync.dma_start(out=wpv_sb[:], in_=wpv[:])
            wqi_sb = cpool.tile([IN_DIM, 2 * D], f32)
            nc.sync.dma_start(out=wqi_sb[:], in_=wqi[:])
            wl_sb = cpool.tile([D, D], bf16)
            nc.sync.dma_start(out=wl_sb[:], in_=wl[:])
            wg_sb = cpool.tile([D, D], bf16)
            nc.sync.dma_start(out=wg_sb[:], in_=wg[:])
            bq_sb = cpool.tile([P, D], f32)
            nc.sync.dma_start(out=bq_sb[:], in_=bq_rep[:])
            ball_sb = cpool.tile([P, D], f32)
            nc.sync.dma_start(out=ball_sb[:], in_=ball_rep[:])
            gwbo_sb = cpool.tile([P, 2 * D], f32)
            nc.sync.dma_start(out=gwbo_sb[:], in_=gwbo[:])
            idb_sb = cpool.tile([P, P], bf16)
            nc.sync.dma_start(out=idb_sb[:], in_=id_bf[:])
            idf_sb = cpool.tile([P, P], f32)
            nc.sync.dma_start(out=idf_sb[:], in_=id_f32[:])

            # PE cold-start priming: each PE instruction supports only ONE
            # sync-wait slot (walrus S3_LW), so make PE observe every
            # DMA-queue semaphore it will depend on, one at a time.
            with tc.tile_pool(name="prime_ps", bufs=1, space="PSUM") as prps:
                pr = prps.tile([1, 2], f32)
                nc.tensor.ldweights(weights=idb_sb[:, 0:1])
                nc.tensor.ldweights(weights=wkv_sb[:, 0:1])
                nc.tensor.ldweights(weights=wpv_sb[:, 0:1])
                nc.tensor.ldweights(weights=wl_sb[:, 0:1])
                nc.tensor.ldweights(weights=wg_sb[:, 0:1])
                nc.tensor.matmul(
                    out=pr[0:1, 0:1], lhsT=idf_sb[:, 0:1], rhs=idf_sb[:, 0:1],
                    start=True, stop=True,
                )
                nc.tensor.matmul(
                    out=pr[0:1, 1:2], lhsT=wqi_sb[:, 0:1], rhs=wqi_sb[:, 0:1],
                    start=True, stop=True,
                )

            for t in range(TILES):
                row0 = _tile_rows(t)
                icol0 = t * (K + 1)

                # ---- q / residual path (f32) ----
                gn = gqp.tile([P, IN_DIM], f32, tag="gn")
                nc.gpsimd.indirect_dma_start(
                    out=gn[:],
                    out_offset=None,
                    in_=feats_f32[:],
                    in_offset=bass.IndirectOffsetOnAxis(
                        ap=idx_sb[:, icol0 + K : icol0 + K + 1], axis=0
                    ),
                )
                gnt_ps = trps.tile([IN_DIM, P], f32, tag="tr")
                nc.tensor.transpose(out=gnt_ps[:], in_=gn[:], identity=idf_sb[:])
                gnt = gaugp.tile([IN_DIM, P], f32, tag="gnt")
                nc.scalar.copy(out=gnt[:], in_=gnt_ps[:])
                qres = qresps.tile([P, 2 * D], f32)
                nc.tensor.matmul(
                    out=qres[:], lhsT=gnt[:], rhs=wqi_sb[:], start=True, stop=True
                )
                q_bf = smp.tile([P, D], bf16, tag="qbf")
                nc.vector.tensor_add(out=q_bf[:], in0=qres[:, 0:D], in1=bq_sb[:])

                # ---- gather + project kp/vvp per neighbor ----
                kpv_sb = kpvsbp.tile([P, K, 2 * D], bf16)
                pps = gaugp.tile([4, K * P], bf16, tag="pps")
                nc.sync.dma_start(out=pps[:], in_=ppfs_t[t, :, :])
                for j in range(K // 4):  # 8 quads
                    quad_ps = trps.tile([IN_DIM, 4 * P], bf16, tag="tr")
                    for jj in range(4):
                        k = 4 * j + jj
                        gq = gqp.tile([P, IN_DIM], bf16, tag="gq")
                        nc.gpsimd.indirect_dma_start(
                            out=gq[:],
                            out_offset=None,
                            in_=feats_bf[:],
                            in_offset=bass.IndirectOffsetOnAxis(
                                ap=idx_sb[:, icol0 + k : icol0 + k + 1], axis=0
                            ),
                        )
                        nc.tensor.transpose(
                            out=quad_ps[:, jj * P : (jj + 1) * P],
                            in_=gq[:],
                            identity=idb_sb[:],
                        )
                    gq4 = gaugp.tile([IN_DIM, 4 * P], bf16, tag="gaug")
                    nc.scalar.copy(out=gq4[:], in_=quad_ps[:])
                    for jj in range(0, 4, 2):
                        kpv_ps = kpvps.tile([P, 4 * D], f32)
                        for u in range(2):
                            k = 4 * j + jj + u
                            nc.tensor.matmul(
                                out=kpv_ps[:, u * 2 * D : (u + 1) * 2 * D],
                                lhsT=gq4[:, (jj + u) * P : (jj + u + 1) * P],
                                rhs=wkv_sb[:],
                                start=True,
                                stop=False,
                            )
                            nc.tensor.matmul(
                                out=kpv_ps[:, u * 2 * D : (u + 1) * 2 * D],
                                lhsT=pps[:, k * P : (k + 1) * P],
                                rhs=wpv_sb[:],
                                start=False,
                                stop=True,
                            )
                        k = 4 * j + jj
                        nc.scalar.copy(
                            out=kpv_sb[:, k : k + 2, :].rearrange("p a b -> p (a b)"),
                            in_=kpv_ps[:],
                        )

                # ---- attention core (DVE) ----
                kp_v = kpv_sb[:, :, 0:D]                       # [P, 32, 128]
                prod1 = prodp.tile([P, K * D], bf16, tag="prod")
                nc.vector.tensor_mul(
                    out=prod1[:].rearrange("p (k d) -> p k d", k=K),
                    in0=kp_v,
                    in1=q_bf[:].unsqueeze(1).to_broadcast([P, K, D]),
                )
                s = smp.tile([P, K * H], f32, tag="s")
                nc.vector.tensor_reduce(
                    out=s[:],
                    in_=prod1[:].rearrange("p (kh c) -> p kh c", c=DH),
                    axis=AX.X,
                    op=ALU.add,
                )
                exps = smp.tile([P, K * H], bf16, tag="exps")
                nc.scalar.activation(out=exps[:], in_=s[:], func=ACT_F.Exp)
                den = smp.tile([P, H], f32, tag="den")
                nc.vector.tensor_reduce(
                    out=den[:],
                    in_=exps[:].rearrange("p (k h) -> p h k", k=K),
                    axis=AX.X,
                    op=ALU.add,
                )
                den_r = smp.tile([P, H], f32, tag="denr")
                nc.vector.reciprocal(out=den_r[:], in_=den[:])

                vvp_v = kpv_sb[:, :, D : 2 * D].rearrange(
                    "p k (h c) -> p k h c", h=H
                )                                               # [P, 32, 8, 16]
                prod2 = prodp.tile([P, K * D], bf16, tag="prod2")
                nc.vector.tensor_mul(
                    out=prod2[:].rearrange("p (k h c) -> p k h c", k=K, h=H),
                    in0=vvp_v,
                    in1=exps[:]
                    .rearrange("p (k h) -> p k h", k=K)
                    .unsqueeze(3)
                    .to_broadcast([P, K, H, DH]),
                )
                hid_u = postp.tile([P, D], f32, tag="hidu")
                nc.vector.tensor_reduce(
                    out=hid_u[:],
                    in_=prod2[:].rearrange("p (k hc) -> p hc k", k=K),
                    axis=AX.X,
                    op=ALU.add,
                )
                hid_bf = postp.tile([P, D], bf16, tag="hidbf")
                nc.vector.tensor_mul(
                    out=hid_bf[:].rearrange("p (h c) -> p h c", h=H),
                    in0=hid_u[:].rearrange("p (h c) -> p h c", h=H),
                    in1=den_r[:].unsqueeze(2).to_broadcast([P, H, DH]),
                )

                # ---- x = hidden@Wl + resid + ball ; LN folded ----
                ht_ps = trps.tile([P, P], bf16, tag="tr")
                nc.tensor.transpose(out=ht_ps[:], in_=hid_bf[:], identity=idb_sb[:])
                ht = postp.tile([P, D], bf16, tag="ht")
                nc.scalar.copy(out=ht[:], in_=ht_ps[:])
                nc.tensor.matmul(
                    out=qres[:, D : 2 * D], lhsT=ht[:], rhs=wl_sb[:],
                    start=False, stop=True,
                )
                x_sb = postp.tile([P, D], bf16, tag="xsb")
                xsum = smp.tile([P, 1], f32, tag="xsum")
                nc.vector.scalar_tensor_tensor(
                    out=x_sb[:],
                    in0=qres[:, D : 2 * D],
                    scalar=0.0,
                    in1=ball_sb[:],
                    op0=ALU.add,
                    op1=ALU.add,
                    accum_out=xsum[:],
                )
                sq_scr = postp.tile([P, D], bf16, tag="sqscr")
                sumsq = smp.tile([P, 1], f32, tag="sumsq")
                nc.scalar.activation(
                    out=sq_scr[:], in_=x_sb[:], func=ACT_F.Square,
                    accum_out=sumsq[:],
                )
                mu_n = smp.tile([P, 1], f32, tag="mun")
                nc.vector.tensor_scalar_mul(out=mu_n[:], in0=xsum[:], scalar1=-1.0 / D)
                e2 = smp.tile([P, 1], f32, tag="e2")
                nc.vector.tensor_scalar_mul(out=e2[:], in0=sumsq[:], scalar1=1.0 / D)
                var = smp.tile([P, 1], f32, tag="var")
                mu2 = smp.tile([P, 1], f32, tag="mu2")
                nc.vector.tensor_mul(out=mu2[:], in0=mu_n[:], in1=mu_n[:])
                # var = (e2 + EPS) - mu^2
                nc.vector.scalar_tensor_tensor(
                    out=var[:], in0=e2[:], scalar=EPS, in1=mu2[:],
                    op0=ALU.add, op1=ALU.subtract,
                )
                sd = smp.tile([P, 1], f32, tag="sd")
                nc.scalar.activation(out=sd[:], in_=var[:], func=ACT_F.Sqrt)
                rs = smp.tile([P, 1], f32, tag="rs")
                nc.vector.reciprocal(out=rs[:], in_=sd[:])
                t_n = smp.tile([P, 1], f32, tag="tn")
                nc.vector.tensor_mul(out=t_n[:], in0=rs[:], in1=mu_n[:])

                xt_ps = trps.tile([P, P], bf16, tag="tr")
                nc.tensor.transpose(out=xt_ps[:], in_=x_sb[:], identity=idb_sb[:])
                xt = postp.tile([P, D], bf16, tag="xt")
                nc.scalar.copy(out=xt[:], in_=xt_ps[:])
                y_ps = yps.tile([P, D], f32)
                nc.tensor.matmul(
                    out=y_ps[:], lhsT=xt[:], rhs=wg_sb[:], start=True, stop=True
                )
                o2 = postp.tile([P, D], f32, tag="o2")
                nc.vector.scalar_tensor_tensor(
                    out=o2[:], in0=gwbo_sb[:, 0:D], scalar=t_n[:],
                    in1=gwbo_sb[:, D : 2 * D], op0=ALU.mult, op1=ALU.add,
                )
                out_sb = postp.tile([P, D], f32, tag="outsb")
                nc.vector.scalar_tensor_tensor(
                    out=out_sb[:], in0=y_ps[:], scalar=rs[:], in1=o2[:],
                    op0=ALU.mult, op1=ALU.add,
                )
                nc.sync.dma_start(out=out[row0 : row0 + P, :], in_=out_sb[:])

    if not nc.is_finalized():
        nc.finalize()
    _BUILD_CACHE["nc"] = nc
    return nc


def _fold_params(inp):
    f = lambda a: np.asarray(a, np.float64)
    W_embed, W_in = f(inp["W_embed"]), f(inp["W_in"])
    b_embed, b_in = f(inp["b_embed"]), f(inp["b_in"])
    Wq, bq = f(inp["Wq"]), f(inp["bq"])
    Wk = f(inp["Wk"])
    Wv, bv = f(inp["Wv"]), f(inp["bv"])
    Wp = f(inp["Wp"])
    Wvp, bvp = f(inp["Wvp"]), f(inp["bvp"])
    Wl, bl = f(inp["Wl"]), f(inp["bl"])
    gamma, beta = f(inp["gamma"]), f(inp["beta"])
    Wout, bout = f(inp["Wout"]), f(inp["bout"])

    scale = 1.0 / np.sqrt(DH)
    Wq_f = (W_in @ Wq) * scale
    bq_f = (b_in @ Wq + bq) * scale
    Wk_f = W_in @ Wk
    Wv_f = W_in @ Wv
    Wp_f = W_embed @ Wp
    Wvp_f = W_embed @ Wvp
    vvp_bias = (b_in @ Wv + bv) + (b_embed @ Wvp + bvp)
    ball = b_in + bl + vvp_bias @ Wl
    Wg = gamma[:, None] * Wout
    gw = gamma @ Wout
    bo = beta @ Wout + bout

    wkv = np.concatenate([Wk_f, Wv_f], 1)
    wpv = np.concatenate([Wp_f, Wvp_f], 1)
    wqi = np.concatenate([Wq_f, W_in], 1)
    return {
        "wkv": wkv.astype(BF16),
        "wpv": wpv.astype(BF16),
        "wqi": wqi.astype(np.float32),
        "wl": Wl.astype(BF16),
        "wg": Wg.astype(BF16),
        "bq_rep": np.tile(bq_f.astype(np.float32)[None, :], (P, 1)),
        "ball_rep": np.tile(ball.astype(np.float32)[None, :], (P, 1)),
        "gwbo": np.tile(
            np.concatenate([gw, bo]).astype(np.float32)[None, :], (P, 1)
        ),
    }


def _make_in_maps(inputs, folded):
    feats = np.asarray(inputs["feats"], np.float32)
    node_idx = np.asarray(inputs["node_idx"], np.int64).astype(np.int32)
    group_idx = np.asarray(inputs["group_idx"], np.int64).astype(np.int32)
    ppfs = np.asarray(inputs["ppfs"], np.float32)

    feats_bf = feats.astype(BF16)
    id_bf = np.eye(P, dtype=BF16)
    id_f32 = np.eye(P, dtype=np.float32)

    in_maps = []
    for c in range(NCORES):
        m0 = c * MS
        rows = np.empty((TILES, P), np.int64)
        for t in range(TILES):
            rows[t] = m0 + _tile_rows(t) + np.arange(P)
        # gidx: [P, TILES*(K+1)] int32, cols t*(K+1)+k
        gidx = np.empty((P, NIDX), np.int32)
        for t in range(TILES):
            gidx[:, t * (K + 1) : t * (K + 1) + K] = group_idx[rows[t], :]
            gidx[:, t * (K + 1) + K] = node_idx[rows[t]]
        # ppfs_t: [TILES, 4, K*P] bf16 : [t, c, k*P + q] = ppfs[row, k, c]
        pp = ppfs[rows.reshape(-1)].reshape(TILES, P, K, 4)
        ppfs_t = np.ascontiguousarray(pp.transpose(0, 3, 2, 1)).reshape(
            TILES, 4, K * P
        )
        im = {
            "feats_bf": feats_bf,
            "feats_f32": feats,
            "gidx": gidx,
            "ppfs_t": ppfs_t.astype(BF16),
            "id_bf": id_bf,
            "id_f32": id_f32,
        }
        im.update(folded)
        in_maps.append(im)
    return in_maps


def kernel(**inputs):
    nc = _build_nc()
    folded = _fold_params(inputs)
    in_maps = _make_in_maps(inputs, folded)
    res = run_bass_kernel_spmd(nc, in_maps, list(range(NCORES)))
    out = np.concatenate(
        [np.asarray(res.results[c]["out"], np.float32) for c in range(NCORES)], 0
    )
    return out



# revision 12
# speedup vs baseline: 1.1258x; 1.1258x over previous
"""Trainium2 Bass kernel for LocalPPFTransformer (sparse attention).

Strategy (data-parallel over M across 8 cores, feats replicated):
  Host folds every pre-attention linear op:
    k = feats@(W_in@Wk), v = feats@(W_in@Wv), q = feats@(W_in@Wq)*0.25
    p = ppfs@(W_embed@Wp), vp = ppfs@(W_embed@Wvp)
  Key/positional biases drop out of softmax (constant per head); value-side
  biases pass through softmax (sum attn = 1) and fold into the x bias.
  LayerNorm folds into y = x@diag(gamma)@Wout with per-row rescale.

  Device per 128-query tile:
    - batched dma_gather (SWDGE gather ucode) of all 33*128 feats rows in
      TWO passes (int16 idx limit 32767 -> two source banks, each with a
      zero row at index 0; masked slots gather zeros; the pass outputs are
      summed for free by PSUM-accumulated pair transposes on PE)
    - pair (2-neighbor) transposes + block-diagonal [128,512] kv matmuls
      and [8,512] positional matmuls accumulate kp/vvp in PSUM
    - Act evacuates PSUM (f32->bf16) into h-major kp and transposed vvp
      layouts so both DVE products hit the 2x bf16 fast path
    - DVE attention core: 2x muls + partial bf16 reduction trees,
      softmax without max subtraction (|scores| << 1)
    - folded LN + output matmul, DMA out
"""

import numpy as np
import ml_dtypes

import concourse.bass as bass
import concourse.bacc as bacc
import concourse.tile as tile
from concourse import mybir
from concourse import library_config
from concourse.bass_utils import run_bass_kernel_spmd

BF16 = ml_dtypes.bfloat16

N, M, K = 50000, 20000, 32
IN_DIM, D, OUT_DIM, H = 64, 128, 128, 8
DH = D // H
EPS = 1e-5
NCORES = 8
MS = M // NCORES          # 2500 queries per core
P = 128                   # partitions / tile query count
TILES = (MS + P - 1) // P  # 20 tiles (last overlaps)
NSLOT = K + 1             # 32 neighbors + 1 node per query
NI = NSLOT * P            # idx count per tile (4224)
NIC = NI // 16            # idx columns in wrapped layout (264)
BANK1 = 32768             # compacted table rows per half-core segment
PAD = P                   # feats rows padded to 128 cols (256B elems)

_BUILD_CACHE = {}


def _tile_rows(t):
    start = t * P
    if start + P > MS:
        start = MS - P
    return start


def _build_nc():
    if "nc" in _BUILD_CACHE:
        return _BUILD_CACHE["nc"]

    f32 = mybir.dt.float32
    bf16 = mybir.dt.bfloat16
    i16 = mybir.dt.int16

    nc = bacc.Bacc()

    srcT = nc.declare_dram_parameter("srcT", [2 * BANK1, PAD], bf16, isOutput=False)
    gidx = nc.declare_dram_parameter("gidx", [TILES, P, NIC], i16, isOutput=False)
    # ppfs transposed: [tile, 4 coords, K*P]
    ppfs_t = nc.declare_dram_parameter("ppfs_t", [TILES, 4, K * P], bf16, isOutput=False)
    wkvp = nc.declare_dram_parameter("wkvp", [IN_DIM + 4, 2 * D], bf16, isOutput=False)
    wqi = nc.declare_dram_parameter("wqi", [IN_DIM + 4, 2 * D], bf16, isOutput=False)
    wl = nc.declare_dram_parameter("wl", [D, D], bf16, isOutput=False)
    wg = nc.declare_dram_parameter("wg", [D, D], bf16, isOutput=False)
    bq_rep = nc.declare_dram_parameter("bq_rep", [P, D], f32, isOutput=False)
    ball_rep = nc.declare_dram_parameter("ball_rep", [P, D], f32, isOutput=False)
    gwbo = nc.declare_dram_parameter("gwbo", [P, 2 * D], f32, isOutput=False)
    id_bf = nc.declare_dram_parameter("id_bf", [P, P], bf16, isOutput=False)
    out = nc.declare_dram_parameter("out", [MS, OUT_DIM], f32, isOutput=True)

    AX = mybir.AxisListType
    ALU = mybir.AluOpType
    ACT_F = mybir.ActivationFunctionType

    with tile.TileContext(nc) as tc:
        with (
            tc.tile_pool(name="const", bufs=1) as cpool,
            tc.tile_pool(name="idx", bufs=2) as idxp,
            tc.tile_pool(name="gbuf", bufs=2) as gbufp,
            tc.tile_pool(name="gq4", bufs=3) as gq4p,
            tc.tile_pool(name="kpv_sb", bufs=2) as kpvsbp,
            tc.tile_pool(name="prod", bufs=2) as prodp,
            tc.tile_pool(name="attn_sm", bufs=2) as smp,
            tc.tile_pool(name="post", bufs=2) as postp,
            tc.tile_pool(name="tr_ps", bufs=2, space="PSUM") as trps,
            tc.tile_pool(name="kpv_ps", bufs=1, space="PSUM") as kpvps,
            tc.tile_pool(name="qres_ps", bufs=2, space="PSUM") as qresps,
        ):
            nc.gpsimd.load_library(library_config.mlp)

            # ---- static loads ----
            wkv_sb = cpool.tile([IN_DIM + 4, 2 * D], bf16)
            nc.sync.dma_start(out=wkv_sb[:], in_=wkvp[:])
            wqi_sb = cpool.tile([IN_DIM + 4, 2 * D], bf16)
            nc.sync.dma_start(out=wqi_sb[:], in_=wqi[:])
            wl_sb = cpool.tile([D, D], bf16)
            nc.sync.dma_start(out=wl_sb[:], in_=wl[:])
            wg_sb = cpool.tile([D, D], bf16)
            nc.sync.dma_start(out=wg_sb[:], in_=wg[:])
            bq_sb = cpool.tile([P, D], f32)
            nc.sync.dma_start(out=bq_sb[:], in_=bq_rep[:])
            ball_sb = cpool.tile([P, D], f32)
            nc.sync.dma_start(out=ball_sb[:], in_=ball_rep[:])
            gwbo_sb = cpool.tile([P, 2 * D], f32)
            nc.sync.dma_start(out=gwbo_sb[:], in_=gwbo[:])
            idb_sb = cpool.tile([P, P], bf16)
            nc.sync.dma_start(out=idb_sb[:], in_=id_bf[:])

            # PE cold-start priming: each PE instruction supports only ONE
            # sync-wait slot, so make PE observe every DMA-queue semaphore
            # it will depend on, one at a time.
            if True:
                pr = trps.tile([1, 1], f32, tag="tr")
                nc.tensor.ldweights(weights=idb_sb[:, 0:1])
                nc.tensor.ldweights(weights=wkv_sb[:, 0:1])
                nc.tensor.ldweights(weights=wqi_sb[:, 0:1])
                nc.tensor.ldweights(weights=wl_sb[:, 0:1])
                nc.tensor.ldweights(weights=wg_sb[:, 0:1])
                nc.tensor.matmul(
                    out=pr[0:1, 0:1], lhsT=idb_sb[:, 0:1], rhs=idb_sb[:, 0:1],
                    start=True, stop=True,
                )

            for t in range(TILES):
                row0 = _tile_rows(t)

                # ---- single-pass chunked gathers (1024 idx max/call) ----
                seg = 0 if t < TILES // 2 else 1
                idx_sb = idxp.tile([P, NIC], i16, tag="idx")
                nc.sync.dma_start(out=idx_sb[:], in_=gidx[t, :, :])
                gsrc = srcT[seg * BANK1 : (seg + 1) * BANK1, :]
                gws = []
                for w in range(4):
                    gw = gbufp.tile([P, 8, PAD], bf16, tag=f"gw{w}")
                    nc.gpsimd.dma_gather(
                        out_ap=gw[:],
                        in_ap=gsrc,
                        idxs_ap=idx_sb[:, w * 64 : (w + 1) * 64],
                        num_idxs=8 * P,
                        num_idxs_reg=8 * P,
                        elem_size=PAD,
                    )
                    gws.append(gw)
                gn = gbufp.tile([P, 1, PAD], bf16, tag="gn")
                nc.gpsimd.dma_gather(
                    out_ap=gn[:],
                    in_ap=gsrc,
                    idxs_ap=idx_sb[:, 4 * 64 : 4 * 64 + 8],
                    num_idxs=P,
                    num_idxs_reg=P,
                    elem_size=PAD,
                )

                # ---- node slot: transpose + q/resid matmul ----
                ntr = trps.tile([IN_DIM, P], f32, tag="tr")
                nc.tensor.matmul(
                    out=ntr[:], lhsT=gn[:, 0, 0:IN_DIM],
                    rhs=idb_sb[:], start=True, stop=True,
                )
                gnt = gq4p.tile([IN_DIM, P], bf16, tag="gnt")
                nc.scalar.copy(out=gnt[:], in_=ntr[:])
                qres = qresps.tile([P, 2 * D + D], f32)
                nc.tensor.matmul(
                    out=qres[:, 0 : 2 * D], lhsT=gnt[:],
                    rhs=wqi_sb[0:IN_DIM, :], start=True, stop=True,
                )
                q_bf = smp.tile([P, D], bf16, tag="qbf")
                nc.vector.tensor_add(out=q_bf[:], in0=qres[:, 0:D], in1=bq_sb[:])

                # ---- per-neighbor transposes + fused [68,256] projections ----
                kpsb = kpvsbp.tile([P, K, D], bf16, tag="kpsb")
                vvpt = kpvsbp.tile([P, H, DH, K], bf16, tag="vvpt")
                for w in range(4):  # 8-neighbor waves
                    kpv_ps = kpvps.tile([P, 8 * 2 * D], f32)
                    for g in range(2):  # 4-neighbor transpose groups
                        tr4 = trps.tile([IN_DIM, 4 * P], f32, tag="tr")
                        for u in range(4):
                            nc.tensor.matmul(
                                out=tr4[:, u * P : (u + 1) * P],
                                lhsT=gws[w][:, 4 * g + u, 0:IN_DIM],
                                rhs=idb_sb[:], start=True, stop=True,
                            )
                        gq4 = gq4p.tile([IN_DIM + 4, 4 * P], bf16, tag="gq4")
                        # ppfs coords land in contraction rows 64:68 (SBUF)
                        nc.sync.dma_start(
                            out=gq4[IN_DIM : IN_DIM + 4, :],
                            in_=ppfs_t[t, :, (8 * w + 4 * g) * P : (8 * w + 4 * g + 4) * P],
                        )
                        nc.scalar.copy(out=gq4[0:IN_DIM, :], in_=tr4[:])
                        for u in range(4):
                            uu = 4 * g + u
                            nc.tensor.matmul(
                                out=kpv_ps[:, uu * 2 * D : (uu + 1) * 2 * D],
                                lhsT=gq4[:, u * P : (u + 1) * P],
                                rhs=wkv_sb[:], start=True, stop=True,
                            )
                    # evacuate: kp (k-major) + vvp (transposed)
                    kview = kpv_ps[:].rearrange("p (n x) -> p n x", x=2 * D)
                    nc.scalar.copy(
                        out=kpsb[:, 8 * w : 8 * w + 8, :],
                        in_=kview[:, :, 0:D],
                    )
                    nc.scalar.copy(
                        out=vvpt[:, :, :, 8 * w : 8 * w + 8].rearrange(
                            "p h c n -> p n (h c)"
                        ),
                        in_=kview[:, :, D : 2 * D],
                    )

                # ---- attention core (DVE, h-major) ----
                prod1 = prodp.tile([P, H, K, DH], bf16, tag="prod1")
                nc.vector.tensor_mul(
                    out=prod1[:],
                    in0=kpsb[:].rearrange("p k (h c) -> p h k c", h=H),
                    in1=q_bf[:]
                    .rearrange("p (h c) -> p h c", h=H)
                    .unsqueeze(2)
                    .to_broadcast([P, H, K, DH]),
                )
                st1 = prodp.tile([P, H, K, DH // 2], bf16, tag="st1")
                nc.vector.tensor_add(
                    out=st1[:], in0=prod1[:, :, :, 0:8], in1=prod1[:, :, :, 8:16]
                )
                st2 = prodp.tile([P, H, K, DH // 4], bf16, tag="st2")
                nc.vector.tensor_add(
                    out=st2[:], in0=st1[:, :, :, 0:4], in1=st1[:, :, :, 4:8]
                )
                s = smp.tile([P, H * K], f32, tag="s")
                nc.vector.tensor_reduce(
                    out=s[:],
                    in_=st2[:].rearrange("p h k c -> p (h k) c"),
                    axis=AX.X,
                    op=ALU.add,
                )
                exps = smp.tile([P, H, K], bf16, tag="exps")
                nc.scalar.activation(
                    out=exps[:].rearrange("p h k -> p (h k)"), in_=s[:],
                    func=ACT_F.Exp,
                )
                den = smp.tile([P, H], f32, tag="den")
                nc.vector.tensor_reduce(
                    out=den[:], in_=exps[:], axis=AX.X, op=ALU.add
                )
                den_r = smp.tile([P, H], f32, tag="denr")
                nc.vector.reciprocal(out=den_r[:], in_=den[:])

                prod2 = prodp.tile([P, H, DH, K], bf16, tag="prod2")
                nc.vector.tensor_mul(
                    out=prod2[:],
                    in0=vvpt[:],
                    in1=exps[:].unsqueeze(2).to_broadcast([P, H, DH, K]),
                )
                ht1 = prodp.tile([P, H, DH, K // 2], bf16, tag="ht1")
                nc.vector.tensor_add(
                    out=ht1[:], in0=prod2[:, :, :, 0:16], in1=prod2[:, :, :, 16:32]
                )
                ht2 = prodp.tile([P, H, DH, K // 4], bf16, tag="ht2")
                nc.vector.tensor_add(
                    out=ht2[:], in0=ht1[:, :, :, 0:8], in1=ht1[:, :, :, 8:16]
                )
                ht3 = prodp.tile([P, H, DH, K // 8], bf16, tag="ht3")
                nc.vector.tensor_add(
                    out=ht3[:], in0=ht2[:, :, :, 0:4], in1=ht2[:, :, :, 4:8]
                )
                hid_u = postp.tile([P, D], f32, tag="hidu")
                nc.vector.tensor_reduce(
                    out=hid_u[:],
                    in_=ht3[:].rearrange("p h c k -> p (h c) k"),
                    axis=AX.X,
                    op=ALU.add,
                )
                hid_bf = postp.tile([P, D], bf16, tag="hidbf")
                nc.vector.tensor_mul(
                    out=hid_bf[:].rearrange("p (h c) -> p h c", h=H),
                    in0=hid_u[:].rearrange("p (h c) -> p h c", h=H),
                    in1=den_r[:].unsqueeze(2).to_broadcast([P, H, DH]),
                )

                # ---- x = hidden@Wl + resid + ball ; LN folded ----
                ht_ps = trps.tile([P, P], f32, tag="tr")
                nc.tensor.matmul(
                    out=ht_ps[:], lhsT=hid_bf[:], rhs=idb_sb[:],
                    start=True, stop=True,
                )
                ht = postp.tile([P, D], bf16, tag="ht")
                nc.scalar.copy(out=ht[:], in_=ht_ps[:])
                nc.tensor.matmul(
                    out=qres[:, D : 2 * D], lhsT=ht[:], rhs=wl_sb[:],
                    start=False, stop=True,
                )
                x_sb = postp.tile([P, D], bf16, tag="xsb")
                xsum = smp.tile([P, 1], f32, tag="xsum")
                nc.vector.scalar_tensor_tensor(
                    out=x_sb[:],
                    in0=qres[:, D : 2 * D],
                    scalar=0.0,
                    in1=ball_sb[:],
                    op0=ALU.add,
                    op1=ALU.add,
                    accum_out=xsum[:],
                )
                sq_scr = postp.tile([P, D], bf16, tag="sqscr")
                sumsq = smp.tile([P, 1], f32, tag="sumsq")
                nc.scalar.activation(
                    out=sq_scr[:], in_=x_sb[:], func=ACT_F.Square,
                    accum_out=sumsq[:],
                )
                mu_n = smp.tile([P, 1], f32, tag="mun")
                nc.vector.tensor_scalar_mul(out=mu_n[:], in0=xsum[:], scalar1=-1.0 / D)
                e2 = smp.tile([P, 1], f32, tag="e2")
                nc.vector.tensor_scalar_mul(out=e2[:], in0=sumsq[:], scalar1=1.0 / D)
                var = smp.tile([P, 1], f32, tag="var")
                mu2 = smp.tile([P, 1], f32, tag="mu2")
                nc.vector.tensor_mul(out=mu2[:], in0=mu_n[:], in1=mu_n[:])
                nc.vector.scalar_tensor_tensor(
                    out=var[:], in0=e2[:], scalar=EPS, in1=mu2[:],
                    op0=ALU.add, op1=ALU.subtract,
                )
                sd = smp.tile([P, 1], f32, tag="sd")
                nc.scalar.activation(out=sd[:], in_=var[:], func=ACT_F.Sqrt)
                rs = smp.tile([P, 1], f32, tag="rs")
                nc.vector.reciprocal(out=rs[:], in_=sd[:])
                t_n = smp.tile([P, 1], f32, tag="tn")
                nc.vector.tensor_mul(out=t_n[:], in0=rs[:], in1=mu_n[:])

                xt_ps = trps.tile([P, P], f32, tag="tr")
                nc.tensor.matmul(
                    out=xt_ps[:], lhsT=x_sb[:], rhs=idb_sb[:],
                    start=True, stop=True,
                )
                xt = postp.tile([P, D], bf16, tag="xt")
                nc.scalar.copy(out=xt[:], in_=xt_ps[:])
                nc.tensor.matmul(
                    out=qres[:, 2 * D : 3 * D], lhsT=xt[:], rhs=wg_sb[:],
                    start=True, stop=True,
                )
                o2 = postp.tile([P, D], f32, tag="o2")
                nc.vector.scalar_tensor_tensor(
                    out=o2[:], in0=gwbo_sb[:, 0:D], scalar=t_n[:],
                    in1=gwbo_sb[:, D : 2 * D], op0=ALU.mult, op1=ALU.add,
                )
                out_sb = postp.tile([P, D], f32, tag="outsb")
                nc.vector.scalar_tensor_tensor(
                    out=out_sb[:], in0=qres[:, 2 * D : 3 * D], scalar=rs[:],
                    in1=o2[:], op0=ALU.mult, op1=ALU.add,
                )
                nc.sync.dma_start(out=out[row0 : row0 + P, :], in_=out_sb[:])

    if not nc.is_finalized():
        nc.finalize()
    _BUILD_CACHE["nc"] = nc
    return nc


def _fold_params(inp):
    f = lambda a: np.asarray(a, np.float64)
    W_embed, W_in = f(inp["W_embed"]), f(inp["W_in"])
    b_embed, b_in = f(inp["b_embed"]), f(inp["b_in"])
    Wq, bq = f(inp["Wq"]), f(inp["bq"])
    Wk = f(inp["Wk"])
    Wv, bv = f(inp["Wv"]), f(inp["bv"])
    Wp = f(inp["Wp"])
    Wvp, bvp = f(inp["Wvp"]), f(inp["bvp"])
    Wl, bl = f(inp["Wl"]), f(inp["bl"])
    gamma, beta = f(inp["gamma"]), f(inp["beta"])
    Wout, bout = f(inp["Wout"]), f(inp["bout"])

    scale = 1.0 / np.sqrt(DH)
    Wq_f = (W_in @ Wq) * scale
    bq_f = (b_in @ Wq + bq) * scale
    Wk_f = W_in @ Wk
    Wv_f = W_in @ Wv
    Wp_f = W_embed @ Wp
    Wvp_f = W_embed @ Wvp
    vvp_bias = (b_in @ Wv + bv) + (b_embed @ Wvp + bvp)
    ball = b_in + bl + vvp_bias @ Wl
    Wg = gamma[:, None] * Wout
    gw = gamma @ Wout
    bo = beta @ Wout + bout

    wkv = np.concatenate([Wk_f, Wv_f], 1)          # [64, 256]
    wpv = np.concatenate([Wp_f, Wvp_f], 1)         # [4, 256]
    wkvp = np.concatenate([wkv, wpv], 0)           # [68, 256]
    wqi = np.concatenate([Wq_f, W_in], 1)
    wqi = np.concatenate([wqi, np.zeros((4, 2 * D))], 0)
    return {
        "wkvp": wkvp.astype(BF16),
        "wqi": wqi.astype(BF16),
        "wl": Wl.astype(BF16),
        "wg": Wg.astype(BF16),
        "bq_rep": np.tile(bq_f.astype(np.float32)[None, :], (P, 1)),
        "ball_rep": np.tile(ball.astype(np.float32)[None, :], (P, 1)),
        "gwbo": np.tile(
            np.concatenate([gw, bo]).astype(np.float32)[None, :], (P, 1)
        ),
    }


def _make_in_maps(inputs, folded):
    feats = np.asarray(inputs["feats"], np.float32)
    node_idx = np.asarray(inputs["node_idx"], np.int64).astype(np.int32)
    group_idx = np.asarray(inputs["group_idx"], np.int64).astype(np.int32)
    ppfs = np.asarray(inputs["ppfs"], np.float32)

    feats_pad = np.zeros((N, PAD), BF16)
    feats_pad[:, 0:IN_DIM] = feats.astype(BF16)
    id_bf = np.eye(P, dtype=BF16)

    in_maps = []
    for c in range(NCORES):
        m0 = c * MS
        rows = np.empty((TILES, P), np.int64)
        for t in range(TILES):
            rows[t] = m0 + _tile_rows(t) + np.arange(P)
        # full idx per tile: slot[c_slot, p] at list pos c_slot*128+p
        allidx = np.empty((TILES, NSLOT, P), np.int64)
        for t in range(TILES):
            allidx[t, 0:K, :] = group_idx[rows[t], :].T
            allidx[t, K, :] = node_idx[rows[t]]
        # per half-core segment: compact unique rows into an int16-indexable
        # table; device gathers from the compacted table
        srcT = np.zeros((2 * BANK1, PAD), BF16)
        local = np.empty_like(allidx)
        for s in range(2):
            tl = slice(s * (TILES // 2), (s + 1) * (TILES // 2))
            uniq = np.unique(allidx[tl].ravel())
            assert len(uniq) <= BANK1, len(uniq)
            srcT[s * BANK1 : s * BANK1 + len(uniq)] = feats_pad[uniq]
            local[tl] = np.searchsorted(uniq, allidx[tl])
        # wrapped idx layout per 1024-chunk: chunk w covers slots 8w..8w+8;
        # within chunk, idx j -> [16a + j%16, w*64 + j//16]
        gidx = np.zeros((TILES, P, NIC), np.int16)
        for t in range(TILES):
            for w in range(4):
                arr = local[t, 8 * w : 8 * w + 8, :].reshape(8 * P)
                wr = arr.reshape(8 * P // 16, 16).T.astype(np.int16)
                gidx[t, :, w * 64 : (w + 1) * 64] = np.tile(wr, (8, 1))
            nd = local[t, K, :].reshape(P // 16, 16).T.astype(np.int16)
            gidx[t, :, 4 * 64 : 4 * 64 + 8] = np.tile(nd, (8, 1))
        # ppfs_t: [TILES, 4, K*P] : [t, cc, k*P + q] = ppfs[row, k, cc]
        pp = ppfs[rows.reshape(-1)].reshape(TILES, P, K, 4)
        ppfs_t = np.ascontiguousarray(pp.transpose(0, 3, 2, 1)).reshape(
            TILES, 4, K * P
        )
        im = {
            "srcT": srcT,
            "gidx": gidx,
            "ppfs_t": ppfs_t.astype(BF16),
            "id_bf": id_bf,
        }
        im.update(folded)
        in_maps.append(im)
    return in_maps


def kernel(**inputs):
    nc = _build_nc()
    folded = _fold_params(inputs)
    in_maps = _make_in_maps(inputs, folded)
    res = run_bass_kernel_spmd(nc, in_maps, list(range(NCORES)))
    out = np.concatenate(
        [np.asarray(res.results[c]["out"], np.float32) for c in range(NCORES)], 0
    )
    return out


# revision 13
# speedup vs baseline: 1.1346x; 1.0078x over previous
"""Trainium2 Bass kernel for LocalPPFTransformer (sparse attention).

Strategy (data-parallel over M across 8 cores, feats replicated):
  Host folds every pre-attention linear op:
    k = feats@(W_in@Wk), v = feats@(W_in@Wv), q = feats@(W_in@Wq)*0.25
    p = ppfs@(W_embed@Wp), vp = ppfs@(W_embed@Wvp)
  Key/positional biases drop out of softmax (constant per head); value-side
  biases pass through softmax (sum attn = 1) and fold into the x bias.
  LayerNorm folds into y = x@diag(gamma)@Wout with per-row rescale.

  Device per 128-query tile:
    - batched dma_gather (SWDGE gather ucode) of all 33*128 feats rows in
      TWO passes (int16 idx limit 32767 -> two source banks, each with a
      zero row at index 0; masked slots gather zeros; the pass outputs are
      summed for free by PSUM-accumulated pair transposes on PE)
    - pair (2-neighbor) transposes + block-diagonal [128,512] kv matmuls
      and [8,512] positional matmuls accumulate kp/vvp in PSUM
    - Act evacuates PSUM (f32->bf16) into h-major kp and transposed vvp
      layouts so both DVE products hit the 2x bf16 fast path
    - DVE attention core: 2x muls + partial bf16 reduction trees,
      softmax without max subtraction (|scores| << 1)
    - folded LN + output matmul, DMA out
"""

import numpy as np
import ml_dtypes

import concourse.bass as bass
import concourse.bacc as bacc
import concourse.tile as tile
from concourse import mybir
from concourse import library_config
from concourse.bass_utils import run_bass_kernel_spmd

BF16 = ml_dtypes.bfloat16

N, M, K = 50000, 20000, 32
IN_DIM, D, OUT_DIM, H = 64, 128, 128, 8
DH = D // H
EPS = 1e-5
NCORES = 8
MS = M // NCORES          # 2500 queries per core
P = 128                   # partitions / tile query count
TILES = (MS + P - 1) // P  # 20 tiles (last overlaps)
NSLOT = K + 1             # 32 neighbors + 1 node per query
NI = NSLOT * P            # idx count per tile (4224)
NIC = NI // 16            # idx columns in wrapped layout (264)
BANK1 = 32768             # compacted table rows per half-core segment
PAD = P                   # feats rows padded to 128 cols (256B elems)

_BUILD_CACHE = {}


def _tile_rows(t):
    start = t * P
    if start + P > MS:
        start = MS - P
    return start


def _build_nc():
    if "nc" in _BUILD_CACHE:
        return _BUILD_CACHE["nc"]

    f32 = mybir.dt.float32
    bf16 = mybir.dt.bfloat16
    i16 = mybir.dt.int16

    nc = bacc.Bacc()

    srcT = nc.declare_dram_parameter("srcT", [2 * BANK1, PAD], bf16, isOutput=False)
    gidx = nc.declare_dram_parameter("gidx", [TILES, P, NIC], i16, isOutput=False)
    # ppfs transposed: [tile, 4 coords, K*P]
    ppfs_t = nc.declare_dram_parameter("ppfs_t", [TILES, 4, K * P], bf16, isOutput=False)
    wkvp = nc.declare_dram_parameter("wkvp", [IN_DIM + 4, 2 * D], bf16, isOutput=False)
    wqi = nc.declare_dram_parameter("wqi", [IN_DIM + 4, 2 * D], bf16, isOutput=False)
    wl = nc.declare_dram_parameter("wl", [D, D], bf16, isOutput=False)
    wg = nc.declare_dram_parameter("wg", [D, D], bf16, isOutput=False)
    bq_rep = nc.declare_dram_parameter("bq_rep", [P, D], f32, isOutput=False)
    ball_rep = nc.declare_dram_parameter("ball_rep", [P, D], f32, isOutput=False)
    gwbo = nc.declare_dram_parameter("gwbo", [P, 2 * D], f32, isOutput=False)
    id_bf = nc.declare_dram_parameter("id_bf", [P, P], bf16, isOutput=False)
    out = nc.declare_dram_parameter("out", [MS, OUT_DIM], f32, isOutput=True)

    AX = mybir.AxisListType
    ALU = mybir.AluOpType
    ACT_F = mybir.ActivationFunctionType

    with tile.TileContext(nc) as tc:
        with (
            tc.tile_pool(name="const", bufs=1) as cpool,
            tc.tile_pool(name="idx", bufs=2) as idxp,
            tc.tile_pool(name="gbuf", bufs=2) as gbufp,
            tc.tile_pool(name="gq4", bufs=3) as gq4p,
            tc.tile_pool(name="kpv_sb", bufs=2) as kpvsbp,
            tc.tile_pool(name="prod", bufs=2) as prodp,
            tc.tile_pool(name="attn_sm", bufs=2) as smp,
            tc.tile_pool(name="post", bufs=2) as postp,
            tc.tile_pool(name="tr_ps", bufs=2, space="PSUM") as trps,
            tc.tile_pool(name="kpv_ps", bufs=1, space="PSUM") as kpvps,
            tc.tile_pool(name="qres_ps", bufs=2, space="PSUM") as qresps,
        ):
            nc.gpsimd.load_library(library_config.mlp)

            # ---- static loads ----
            wkv_sb = cpool.tile([IN_DIM + 4, 2 * D], bf16)
            nc.sync.dma_start(out=wkv_sb[:], in_=wkvp[:])
            wqi_sb = cpool.tile([IN_DIM + 4, 2 * D], bf16)
            nc.sync.dma_start(out=wqi_sb[:], in_=wqi[:])
            wl_sb = cpool.tile([D, D], bf16)
            nc.sync.dma_start(out=wl_sb[:], in_=wl[:])
            wg_sb = cpool.tile([D, D], bf16)
            nc.sync.dma_start(out=wg_sb[:], in_=wg[:])
            bq_sb = cpool.tile([P, D], f32)
            nc.sync.dma_start(out=bq_sb[:], in_=bq_rep[:])
            ball_sb = cpool.tile([P, D], f32)
            nc.sync.dma_start(out=ball_sb[:], in_=ball_rep[:])
            gwbo_sb = cpool.tile([P, 2 * D], f32)
            nc.sync.dma_start(out=gwbo_sb[:], in_=gwbo[:])
            idb_sb = cpool.tile([P, P], bf16)
            nc.sync.dma_start(out=idb_sb[:], in_=id_bf[:])

            # PE cold-start priming: each PE instruction supports only ONE
            # sync-wait slot, so make PE observe every DMA-queue semaphore
            # it will depend on, one at a time.
            if True:
                pr = trps.tile([1, 1], f32, tag="tr")
                nc.tensor.ldweights(weights=idb_sb[:, 0:1])
                nc.tensor.ldweights(weights=wkv_sb[:, 0:1])
                nc.tensor.ldweights(weights=wqi_sb[:, 0:1])
                nc.tensor.ldweights(weights=wl_sb[:, 0:1])
                nc.tensor.ldweights(weights=wg_sb[:, 0:1])
                nc.tensor.matmul(
                    out=pr[0:1, 0:1], lhsT=idb_sb[:, 0:1], rhs=idb_sb[:, 0:1],
                    start=True, stop=True,
                )

            for t in range(TILES):
                row0 = _tile_rows(t)

                # ---- single-pass chunked gathers (1024 idx max/call) ----
                seg = 0 if t < TILES // 2 else 1
                idx_sb = idxp.tile([P, NIC], i16, tag="idx")
                nc.sync.dma_start(out=idx_sb[:], in_=gidx[t, :, :])
                gsrc = srcT[seg * BANK1 : (seg + 1) * BANK1, :]
                gws = []
                for w in range(4):
                    gw = gbufp.tile([P, 8, PAD], bf16, tag=f"gw{w}")
                    nc.gpsimd.dma_gather(
                        out_ap=gw[:],
                        in_ap=gsrc,
                        idxs_ap=idx_sb[:, w * 64 : (w + 1) * 64],
                        num_idxs=8 * P,
                        num_idxs_reg=8 * P,
                        elem_size=PAD,
                    )
                    gws.append(gw)
                gn = gbufp.tile([P, 1, PAD], bf16, tag="gn")
                nc.gpsimd.dma_gather(
                    out_ap=gn[:],
                    in_ap=gsrc,
                    idxs_ap=idx_sb[:, 4 * 64 : 4 * 64 + 8],
                    num_idxs=P,
                    num_idxs_reg=P,
                    elem_size=PAD,
                )

                # ---- node slot: transpose + q/resid matmul ----
                ntr = trps.tile([IN_DIM, P], f32, tag="tr")
                nc.tensor.matmul(
                    out=ntr[:], lhsT=gn[:, 0, 0:IN_DIM],
                    rhs=idb_sb[:], start=True, stop=True,
                )
                gnt = gq4p.tile([IN_DIM, P], bf16, tag="gnt")
                nc.vector.tensor_copy(out=gnt[:], in_=ntr[:])
                qres = qresps.tile([P, 2 * D + D], f32)
                nc.tensor.matmul(
                    out=qres[:, 0 : 2 * D], lhsT=gnt[:],
                    rhs=wqi_sb[0:IN_DIM, :], start=True, stop=True,
                )
                q_bf = smp.tile([P, D], bf16, tag="qbf")
                nc.vector.tensor_add(out=q_bf[:], in0=qres[:, 0:D], in1=bq_sb[:])

                # ---- per-neighbor transposes + fused [68,256] projections ----
                kpsb = kpvsbp.tile([P, K, D], bf16, tag="kpsb")
                vvpt = kpvsbp.tile([P, H, DH, K], bf16, tag="vvpt")
                for w in range(4):  # 8-neighbor waves
                    kpv_ps = kpvps.tile([P, 8 * 2 * D], f32)
                    for g in range(2):  # 4-neighbor transpose groups
                        tr4 = trps.tile([IN_DIM, 4 * P], f32, tag="tr")
                        for u in range(4):
                            nc.tensor.matmul(
                                out=tr4[:, u * P : (u + 1) * P],
                                lhsT=gws[w][:, 4 * g + u, 0:IN_DIM],
                                rhs=idb_sb[:], start=True, stop=True,
                            )
                        gq4 = gq4p.tile([IN_DIM + 4, 4 * P], bf16, tag="gq4")
                        # ppfs coords land in contraction rows 64:68 (SBUF)
                        nc.sync.dma_start(
                            out=gq4[IN_DIM : IN_DIM + 4, :],
                            in_=ppfs_t[t, :, (8 * w + 4 * g) * P : (8 * w + 4 * g + 4) * P],
                        )
                        nc.vector.tensor_copy(out=gq4[0:IN_DIM, :], in_=tr4[:])
                        for u in range(4):
                            uu = 4 * g + u
                            nc.tensor.matmul(
                                out=kpv_ps[:, uu * 2 * D : (uu + 1) * 2 * D],
                                lhsT=gq4[:, u * P : (u + 1) * P],
                                rhs=wkv_sb[:], start=True, stop=True,
                            )
                    # evacuate: kp (k-major) + vvp (transposed)
                    kview = kpv_ps[:].rearrange("p (n x) -> p n x", x=2 * D)
                    nc.scalar.copy(
                        out=kpsb[:, 8 * w : 8 * w + 8, :],
                        in_=kview[:, :, 0:D],
                    )
                    nc.scalar.copy(
                        out=vvpt[:, :, :, 8 * w : 8 * w + 8].rearrange(
                            "p h c n -> p n (h c)"
                        ),
                        in_=kview[:, :, D : 2 * D],
                    )

                # ---- attention core (DVE, h-major) ----
                prod1 = prodp.tile([P, H, K, DH], bf16, tag="prod1")
                nc.vector.tensor_mul(
                    out=prod1[:],
                    in0=kpsb[:].rearrange("p k (h c) -> p h k c", h=H),
                    in1=q_bf[:]
                    .rearrange("p (h c) -> p h c", h=H)
                    .unsqueeze(2)
                    .to_broadcast([P, H, K, DH]),
                )
                st1 = prodp.tile([P, H, K, DH // 2], bf16, tag="st1")
                nc.vector.tensor_add(
                    out=st1[:], in0=prod1[:, :, :, 0:8], in1=prod1[:, :, :, 8:16]
                )
                st2 = prodp.tile([P, H, K, DH // 4], bf16, tag="st2")
                nc.vector.tensor_add(
                    out=st2[:], in0=st1[:, :, :, 0:4], in1=st1[:, :, :, 4:8]
                )
                s = smp.tile([P, H * K], f32, tag="s")
                nc.vector.tensor_reduce(
                    out=s[:],
                    in_=st2[:].rearrange("p h k c -> p (h k) c"),
                    axis=AX.X,
                    op=ALU.add,
                )
                exps = smp.tile([P, H, K], bf16, tag="exps")
                nc.scalar.activation(
                    out=exps[:].rearrange("p h k -> p (h k)"), in_=s[:],
                    func=ACT_F.Exp,
                )
                den = smp.tile([P, H], f32, tag="den")
                nc.vector.tensor_reduce(
                    out=den[:], in_=exps[:], axis=AX.X, op=ALU.add
                )
                den_r = smp.tile([P, H], f32, tag="denr")
                nc.vector.reciprocal(out=den_r[:], in_=den[:])

                prod2 = prodp.tile([P, H, DH, K], bf16, tag="prod2")
                nc.vector.tensor_mul(
                    out=prod2[:],
                    in0=vvpt[:],
                    in1=exps[:].unsqueeze(2).to_broadcast([P, H, DH, K]),
                )
                ht1 = prodp.tile([P, H, DH, K // 2], bf16, tag="ht1")
                nc.vector.tensor_add(
                    out=ht1[:], in0=prod2[:, :, :, 0:16], in1=prod2[:, :, :, 16:32]
                )
                ht2 = prodp.tile([P, H, DH, K // 4], bf16, tag="ht2")
                nc.vector.tensor_add(
                    out=ht2[:], in0=ht1[:, :, :, 0:8], in1=ht1[:, :, :, 8:16]
                )
                ht3 = prodp.tile([P, H, DH, K // 8], bf16, tag="ht3")
                nc.vector.tensor_add(
                    out=ht3[:], in0=ht2[:, :, :, 0:4], in1=ht2[:, :, :, 4:8]
                )
                hid_u = postp.tile([P, D], f32, tag="hidu")
                nc.vector.tensor_reduce(
                    out=hid_u[:],
                    in_=ht3[:].rearrange("p h c k -> p (h c) k"),
                    axis=AX.X,
                    op=ALU.add,
                )
                hid_bf = postp.tile([P, D], bf16, tag="hidbf")
                nc.vector.tensor_mul(
                    out=hid_bf[:].rearrange("p (h c) -> p h c", h=H),
                    in0=hid_u[:].rearrange("p (h c) -> p h c", h=H),
                    in1=den_r[:].unsqueeze(2).to_broadcast([P, H, DH]),
                )

                # ---- x = hidden@Wl + resid + ball ; LN folded ----
                ht_ps = trps.tile([P, P], f32, tag="tr")
                nc.tensor.matmul(
                    out=ht_ps[:], lhsT=hid_bf[:], rhs=idb_sb[:],
                    start=True, stop=True,
                )
                ht = postp.tile([P, D], bf16, tag="ht")
                nc.vector.tensor_copy(out=ht[:], in_=ht_ps[:])
                nc.tensor.matmul(
                    out=qres[:, D : 2 * D], lhsT=ht[:], rhs=wl_sb[:],
                    start=False, stop=True,
                )
                x_sb = postp.tile([P, D], bf16, tag="xsb")
                xsum = smp.tile([P, 1], f32, tag="xsum")
                nc.vector.scalar_tensor_tensor(
                    out=x_sb[:],
                    in0=qres[:, D : 2 * D],
                    scalar=0.0,
                    in1=ball_sb[:],
                    op0=ALU.add,
                    op1=ALU.add,
                    accum_out=xsum[:],
                )
                sq_scr = postp.tile([P, D], bf16, tag="sqscr")
                sumsq = smp.tile([P, 1], f32, tag="sumsq")
                nc.scalar.activation(
                    out=sq_scr[:], in_=x_sb[:], func=ACT_F.Square,
                    accum_out=sumsq[:],
                )
                mu_n = smp.tile([P, 1], f32, tag="mun")
                nc.vector.tensor_scalar_mul(out=mu_n[:], in0=xsum[:], scalar1=-1.0 / D)
                e2 = smp.tile([P, 1], f32, tag="e2")
                nc.vector.tensor_scalar_mul(out=e2[:], in0=sumsq[:], scalar1=1.0 / D)
                var = smp.tile([P, 1], f32, tag="var")
                mu2 = smp.tile([P, 1], f32, tag="mu2")
                nc.vector.tensor_mul(out=mu2[:], in0=mu_n[:], in1=mu_n[:])
                nc.vector.scalar_tensor_tensor(
                    out=var[:], in0=e2[:], scalar=EPS, in1=mu2[:],
                    op0=ALU.add, op1=ALU.subtract,
                )
                sd = smp.tile([P, 1], f32, tag="sd")
                nc.scalar.activation(out=sd[:], in_=var[:], func=ACT_F.Sqrt)
                rs = smp.tile([P, 1], f32, tag="rs")
                nc.vector.reciprocal(out=rs[:], in_=sd[:])
                t_n = smp.tile([P, 1], f32, tag="tn")
                nc.vector.tensor_mul(out=t_n[:], in0=rs[:], in1=mu_n[:])

                xt_ps = trps.tile([P, P], f32, tag="tr")
                nc.tensor.matmul(
                    out=xt_ps[:], lhsT=x_sb[:], rhs=idb_sb[:],
                    start=True, stop=True,
                )
                xt = postp.tile([P, D], bf16, tag="xt")
                nc.vector.tensor_copy(out=xt[:], in_=xt_ps[:])
                nc.tensor.matmul(
                    out=qres[:, 2 * D : 3 * D], lhsT=xt[:], rhs=wg_sb[:],
                    start=True, stop=True,
                )
                o2 = postp.tile([P, D], f32, tag="o2")
                nc.vector.scalar_tensor_tensor(
                    out=o2[:], in0=gwbo_sb[:, 0:D], scalar=t_n[:],
                    in1=gwbo_sb[:, D : 2 * D], op0=ALU.mult, op1=ALU.add,
                )
                out_sb = postp.tile([P, D], f32, tag="outsb")
                nc.vector.scalar_tensor_tensor(
                    out=out_sb[:], in0=qres[:, 2 * D : 3 * D], scalar=rs[:],
                    in1=o2[:], op0=ALU.mult, op1=ALU.add,
                )
                nc.sync.dma_start(out=out[row0 : row0 + P, :], in_=out_sb[:])

    if not nc.is_finalized():
        nc.finalize()
    _BUILD_CACHE["nc"] = nc
    return nc


def _fold_params(inp):
    f = lambda a: np.asarray(a, np.float64)
    W_embed, W_in = f(inp["W_embed"]), f(inp["W_in"])
    b_embed, b_in = f(inp["b_embed"]), f(inp["b_in"])
    Wq, bq = f(inp["Wq"]), f(inp["bq"])
    Wk = f(inp["Wk"])
    Wv, bv = f(inp["Wv"]), f(inp["bv"])
    Wp = f(inp["Wp"])
    Wvp, bvp = f(inp["Wvp"]), f(inp["bvp"])
    Wl, bl = f(inp["Wl"]), f(inp["bl"])
    gamma, beta = f(inp["gamma"]), f(inp["beta"])
    Wout, bout = f(inp["Wout"]), f(inp["bout"])

    scale = 1.0 / np.sqrt(DH)
    Wq_f = (W_in @ Wq) * scale
    bq_f = (b_in @ Wq + bq) * scale
    Wk_f = W_in @ Wk
    Wv_f = W_in @ Wv
    Wp_f = W_embed @ Wp
    Wvp_f = W_embed @ Wvp
    vvp_bias = (b_in @ Wv + bv) + (b_embed @ Wvp + bvp)
    ball = b_in + bl + vvp_bias @ Wl
    Wg = gamma[:, None] * Wout
    gw = gamma @ Wout
    bo = beta @ Wout + bout

    wkv = np.concatenate([Wk_f, Wv_f], 1)          # [64, 256]
    wpv = np.concatenate([Wp_f, Wvp_f], 1)         # [4, 256]
    wkvp = np.concatenate([wkv, wpv], 0)           # [68, 256]
    wqi = np.concatenate([Wq_f, W_in], 1)
    wqi = np.concatenate([wqi, np.zeros((4, 2 * D))], 0)
    return {
        "wkvp": wkvp.astype(BF16),
        "wqi": wqi.astype(BF16),
        "wl": Wl.astype(BF16),
        "wg": Wg.astype(BF16),
        "bq_rep": np.tile(bq_f.astype(np.float32)[None, :], (P, 1)),
        "ball_rep": np.tile(ball.astype(np.float32)[None, :], (P, 1)),
        "gwbo": np.tile(
            np.concatenate([gw, bo]).astype(np.float32)[None, :], (P, 1)
        ),
    }


def _make_in_maps(inputs, folded):
    feats = np.asarray(inputs["feats"], np.float32)
    node_idx = np.asarray(inputs["node_idx"], np.int64).astype(np.int32)
    group_idx = np.asarray(inputs["group_idx"], np.int64).astype(np.int32)
    ppfs = np.asarray(inputs["ppfs"], np.float32)

    feats_pad = np.zeros((N, PAD), BF16)
    feats_pad[:, 0:IN_DIM] = feats.astype(BF16)
    id_bf = np.eye(P, dtype=BF16)

    in_maps = []
    for c in range(NCORES):
        m0 = c * MS
        rows = np.empty((TILES, P), np.int64)
        for t in range(TILES):
            rows[t] = m0 + _tile_rows(t) + np.arange(P)
        # full idx per tile: slot[c_slot, p] at list pos c_slot*128+p
        allidx = np.empty((TILES, NSLOT, P), np.int64)
        for t in range(TILES):
            allidx[t, 0:K, :] = group_idx[rows[t], :].T
            allidx[t, K, :] = node_idx[rows[t]]
        # per half-core segment: compact unique rows into an int16-indexable
        # table; device gathers from the compacted table
        srcT = np.zeros((2 * BANK1, PAD), BF16)
        local = np.empty_like(allidx)
        for s in range(2):
            tl = slice(s * (TILES // 2), (s + 1) * (TILES // 2))
            uniq = np.unique(allidx[tl].ravel())
            assert len(uniq) <= BANK1, len(uniq)
            srcT[s * BANK1 : s * BANK1 + len(uniq)] = feats_pad[uniq]
            local[tl] = np.searchsorted(uniq, allidx[tl])
        # wrapped idx layout per 1024-chunk: chunk w covers slots 8w..8w+8;
        # within chunk, idx j -> [16a + j%16, w*64 + j//16]
        gidx = np.zeros((TILES, P, NIC), np.int16)
        for t in range(TILES):
            for w in range(4):
                arr = local[t, 8 * w : 8 * w + 8, :].reshape(8 * P)
                wr = arr.reshape(8 * P // 16, 16).T.astype(np.int16)
                gidx[t, :, w * 64 : (w + 1) * 64] = np.tile(wr, (8, 1))
            nd = local[t, K, :].reshape(P // 16, 16).T.astype(np.int16)
            gidx[t, :, 4 * 64 : 4 * 64 + 8] = np.tile(nd, (8, 1))
        # ppfs_t: [TILES, 4, K*P] : [t, cc, k*P + q] = ppfs[row, k, cc]
        pp = ppfs[rows.reshape(-1)].reshape(TILES, P, K, 4)
        ppfs_t = np.ascontiguousarray(pp.transpose(0, 3, 2, 1)).reshape(
            TILES, 4, K * P
        )
        im = {
            "srcT": srcT,
            "gidx": gidx,
            "ppfs_t": ppfs_t.astype(BF16),
            "id_bf": id_bf,
        }
        im.update(folded)
        in_maps.append(im)
    return in_maps


def kernel(**inputs):
    nc = _build_nc()
    folded = _fold_params(inputs)
    in_maps = _make_in_maps(inputs, folded)
    res = run_bass_kernel_spmd(nc, in_maps, list(range(NCORES)))
    out = np.concatenate(
        [np.asarray(res.results[c]["out"], np.float32) for c in range(NCORES)], 0
    )
    return out
